# revision 22
# baseline (speedup 1.0000x reference)
"""BiaffineAttention TRN2 kernel.

Full-input contract: kernel(**inputs) takes the unsharded reference inputs
(hidden_states [16,512,1024] f32 + MLP/bilinear params) and returns the full
arc_scores [16,512,512] f32.

Strategy:
- Data-parallel over batch across 8 NeuronCores (2 batches/core).
- All on-chip compute is feature-major (arc/hidden on partitions), so every
  matmul has its contraction dim on partitions and there are no on-chip
  transposes: the host passes x pre-transposed per core and weights
  pre-transposed + zero-padded (arc 500 -> 512).
- The bilinear weight Wb is fused into the head MLP's second layer on the
  host (Wf = w2h.T @ Wb, bf = b2h @ Wb, in float64), removing a whole
  [500x500] GEMM stage from the device.
- Matmuls run in float16 (same 10-bit mantissa as tf32, ~5e-4 relative
  error, but 2-byte operands at full PE issue rate with pipelined weight
  loads). float32r/bf16 variants remain selectable via _CACHE for testing.
"""

import sys

if "/opt/trn_rl_repo" not in sys.path:
    sys.path.insert(0, "/opt/trn_rl_repo")

import numpy as np

import concourse.bacc as bacc
import concourse.mybir as mybir
import concourse.tile as tile
from concourse.bass_utils import run_bass_kernel_spmd

N_CORES = 8
BATCH = 16
SEQ = 512
HIDDEN = 1024
ARC = 500
ARC_P = 512  # arc padded to a multiple of 128

P = 128
B_PER_CORE = BATCH // N_CORES  # 2
R = B_PER_CORE * SEQ  # 1024 rows per core
HK = HIDDEN // P  # 8 hidden k-tiles
AK = ARC_P // P  # 4 arc tiles
RC = R // SEQ  # 2 row chunks of 512

F32 = mybir.dt.float32
F32R = mybir.dt.float32r
BF16 = mybir.dt.bfloat16
F16 = mybir.dt.float16
AF = mybir.ActivationFunctionType

# matmul operand dtypes: (stationary/weight side, moving/activation side)
_DT_MODES = {
    "f32r": (F32R, F32R),
    "bf16": (BF16, BF16),
    "fp16": (F16, F16),
    "mixed": (BF16, F32R),
}
# max moving-operand width: 512 for 4-byte dtypes, 1024 for 2-byte
_MOV_W = {"f32r": 512, "bf16": 512, "fp16": 512, "mixed": 512}

_CACHE = {}


_DEFAULTS = {"dt_mode": "fp16"}


def _cfg(name, default):
    return _CACHE.get(name, _DEFAULTS.get(name, default))


def _emit(nc, tc, aps, loop_n=0):
    import contextlib

    mode = _cfg("dt_mode", "f32r")
    sd, md = _DT_MODES[mode]
    # scores-phase dtypes (both operands are on-chip activations)
    ssd = md if _cfg("scores_f32r", True) else sd
    smd = md

    ctx = contextlib.ExitStack()
    with ctx:
        cpool = ctx.enter_context(tc.tile_pool(name="const", bufs=1))
        apool = ctx.enter_context(tc.tile_pool(name="acts", bufs=1))
        pspool = ctx.enter_context(
            tc.tile_pool(name="psum", bufs=_cfg("ps_bufs", 8), space="PSUM")
        )
        opool = ctx.enter_context(tc.tile_pool(name="outs", bufs=8))

        # ---- constant loads, split per k-tile and ordered by first use so the
        # first matmul group starts as soon as its own slices land
        xT = cpool.tile([P, HK, R], md, tag="xT")
        w1h = cpool.tile([P, HK, ARC_P], sd, tag="w1h")
        w1d = cpool.tile([P, HK, ARC_P], sd, tag="w1d")
        xT_src = aps["xT"].rearrange("(ko p) r -> p ko r", p=P)
        w1h_src = aps["w1hT"].rearrange("(ko p) a -> p ko a", p=P)
        w1d_src = aps["w1dT"].rearrange("(ko p) a -> p ko a", p=P)
        for k in range(HK):
            (nc.sync if k == 0 else nc.gpsimd).dma_start(w1h[:, k], w1h_src[:, k])
        for rc in range(RC):
            for k in range(HK):
                eng = nc.scalar if k % 2 == 0 else nc.sync
                eng.dma_start(
                    xT[:, k, rc * SEQ : (rc + 1) * SEQ],
                    xT_src[:, k, rc * SEQ : (rc + 1) * SEQ],
                )
        biases = cpool.tile([P, 4 * AK + 1], F32, tag="biases")
        nc.gpsimd.dma_start(biases[:], aps["biasesL"])
        b1h = biases[:, 0 * AK : 1 * AK]
        bfh = biases[:, 1 * AK : 2 * AK]
        b1d = biases[:, 2 * AK : 3 * AK]
        b2d = biases[:, 3 * AK : 4 * AK]
        for k in range(HK):
            nc.gpsimd.dma_start(w1d[:, k], w1d_src[:, k])
        wf = cpool.tile([P, AK, ARC_P], sd, tag="wf")
        nc.gpsimd.dma_start(wf[:], aps["wfT"].rearrange("(ko p) a -> p ko a", p=P))
        w2d = cpool.tile([P, AK, ARC_P], sd, tag="w2d")
        nc.gpsimd.dma_start(w2d[:], aps["w2dT"].rearrange("(ko p) a -> p ko a", p=P))

        h1h = apool.tile([P, AK, R], md, tag="h1h")
        h1d = apool.tile([P, AK, R], md, tag="h1d")
        headWT = apool.tile([P, AK, R], ssd, tag="headWT")
        depT = apool.tile([P, AK, R], smd, tag="depT")

        def l1_half(w1, b1, h1, rc):
            # h1[:, :, rc-half] = relu(w1.T @ xT + b1); m-pairs share each
            # arriving xT k-tile (2 matmuls per 128KB of DMA feed)
            rs = slice(rc * SEQ, (rc + 1) * SEQ)
            for m0 in range(0, AK, 2):
                psa = pspool.tile([P, SEQ], F32, tag="ps", name=f"l1a_{rc}_{m0}")
                psb = pspool.tile([P, SEQ], F32, tag="ps", name=f"l1b_{rc}_{m0}")
                for k in range(HK):
                    for m, ps in ((m0, psa), (m0 + 1, psb)):
                        nc.tensor.matmul(
                            ps[:],
                            w1[:, k, m * P : (m + 1) * P],
                            xT[:, k, rs],
                            start=(k == 0),
                            stop=(k == HK - 1),
                        )
                for m, ps in ((m0, psa), (m0 + 1, psb)):
                    nc.scalar.activation(
                        h1[:, m, rs], ps[:], AF.Relu, bias=b1[:, m : m + 1]
                    )

        def l2_group(w2, b2, h1, outbuf, rc, m, epi):
            # outbuf[:, m, rc-half] = w2.T @ h1 + b2 at 512 width
            rs = slice(rc * SEQ, (rc + 1) * SEQ)
            ps = pspool.tile([P, SEQ], F32, tag="ps", name=f"ps2l_{rc}_{m}")
            for k in range(AK):
                nc.tensor.matmul(
                    ps[:],
                    w2[:, k, m * P : (m + 1) * P],
                    h1[:, k, rs],
                    start=(k == 0),
                    stop=(k == AK - 1),
                )
            if epi == "dve":
                nc.vector.tensor_tensor(
                    outbuf[:, m, rs],
                    ps[:],
                    b2[:, m : m + 1].to_broadcast((P, SEQ)),
                    mybir.AluOpType.add,
                )
            else:
                nc.scalar.activation(
                    outbuf[:, m, rs], ps[:], AF.Identity, bias=b2[:, m : m + 1]
                )

        def scores_kchunk(b, k, pss):
            # one k-slice of the scores accumulation: needs only the m=k tiles
            # of headWT/depT, so it can sit right after the m=k+1 layer-2 pair
            js = slice(b * SEQ, (b + 1) * SEQ)
            for i in range(AK):
                nc.tensor.matmul(
                    pss[i][:],
                    headWT[:, k, b * SEQ + i * P : b * SEQ + (i + 1) * P],
                    depT[:, k, js],
                    start=(k == 0),
                    stop=(k == AK - 1),
                )

        def scores_out(b, pss):
            # the bilinear bias bb is carried by arc pad row 500
            # (headWT[500,:] = bb via the fused bias, depT[500,:] = 1.0), so
            # the PSUM result is final: plain copy out, alternating engines
            for i in range(AK):
                ot = opool.tile([P, SEQ], F32, tag="scout")
                if i % 2 == 0:
                    nc.vector.tensor_copy(ot[:], pss[i][:])
                else:
                    # Identity (not Copy) keeps the ACT function table unchanged
                    # from the depT bias-adds -- table reloads are expensive
                    nc.scalar.activation(ot[:], pss[i][:], AF.Identity)
                eng = nc.sync if i % 2 == 0 else nc.scalar
                eng.dma_start(aps["scores"][b, i * P : (i + 1) * P, :], ot[:])

        if loop_n:
            hints = _cfg("loop_hints", ())
            if hints == "all":
                hints = tuple(
                    mybir.EngineType(e)
                    for e in ("PE", "Activation", "DVE", "SP", "Pool")
                )
            loop_cm = tc.For_i(0, loop_n, 1, hint_engines=hints)
        else:
            loop_cm = contextlib.nullcontext()
        if _cfg("tiny_body", False) and loop_n:
            with loop_cm:
                tb = apool.tile([P, 16], F32, tag="tinybody")
                nc.vector.tensor_copy(tb[:], biases[:, 0:16])
            return
        with loop_cm:
            # both layer-1s first (independent), so layer-2 never starves PE;
            # rc-halves in DMA-arrival order
            l1_half(w1h, b1h, h1h, 0)
            l1_half(w1d, b1d, h1d, 0)
            l1_half(w1h, b1h, h1h, 1)
            l1_half(w1d, b1d, h1d, 1)
            # layer 2 + scores software-pipelined per batch-half: the scores
            # k-chunk for m=k is emitted after the m=k+1 layer-2 pair, so the
            # in-order PE stream never waits on an epilogue drain.
            for rc in range(B_PER_CORE):
                pss = []
                for i in range(AK):
                    ps = pspool.tile([P, SEQ], F32, tag="ps", name=f"ps2s_{rc}_{i}")
                    pss.append(ps)
                for m in range(AK):
                    l2_group(wf, bfh, h1h, headWT, rc, m, "dve")
                    l2_group(w2d, b2d, h1d, depT, rc, m, "act")
                    if m >= 1:
                        scores_kchunk(rc, m - 1, pss)
                scores_kchunk(rc, AK - 1, pss)
                scores_out(rc, pss)


def _build(loop_n=0):
    sd, md = _DT_MODES[_cfg("dt_mode", "f32r")]
    key = ("nc", _cfg("dt_mode", "f32r"), _cfg("scores_f32r", True), loop_n, _cfg("loop_hints", ()), _cfg("tiny_body", False))
    if key in _CACHE:
        return _CACHE[key]
    nc = bacc.Bacc("TRN2", target_bir_lowering=False, debug=False, num_devices=N_CORES)

    def dram(name, shape, dt):
        return nc.dram_tensor(name, shape, dt, kind="ExternalInput").ap()

    aps = {
        "xT": dram("xT", [HIDDEN, R], md),
        "w1hT": dram("w1hT", [HIDDEN, ARC_P], sd),
        "wfT": dram("wfT", [ARC_P, ARC_P], sd),
        "w1dT": dram("w1dT", [HIDDEN, ARC_P], sd),
        "w2dT": dram("w2dT", [ARC_P, ARC_P], sd),
        "biasesL": dram("biasesL", [P, 4 * AK + 1], F32),
        "scores": nc.dram_tensor(
            "scores", [B_PER_CORE, SEQ, SEQ], F32, kind="ExternalOutput"
        ).ap(),
    }
    with tile.TileContext(nc) as tc:
        _emit(nc, tc, aps, loop_n=loop_n)
    nc.compile()
    _CACHE[key] = nc
    return nc


def _round_tf32(a):
    """fp32 -> tf32 (10-bit mantissa) RNE, returned as fp32 bits."""
    b = np.ascontiguousarray(a, np.float32).view(np.uint32).copy()
    lsb = (b >> 13) & 1
    b += 0x0FFF + lsb
    b &= np.uint32(0xFFFFE000)
    return b.view(np.float32)


def _to_dt(a, dt):
    """Convert fp32 ndarray to the numpy repr of mybir dtype dt."""
    if dt == F32R:
        return _round_tf32(a)
    if dt == BF16:
        import ml_dtypes

        return np.asarray(a, np.float32).astype(ml_dtypes.bfloat16)
    if dt == F16:
        return np.asarray(a, np.float32).astype(np.float16)
    return np.asarray(a, np.float32)


def _bias_layout(b):
    """[ARC] (unpadded) -> [128, AK] with arc index = col*128 + partition."""
    bp = np.zeros(ARC_P, np.float32)
    b = np.asarray(b, np.float32)
    bp[: b.shape[0]] = b
    return np.ascontiguousarray(bp.reshape(AK, P).T)


def _prep_shared(w1h, b1h, w2h, b2h, w1d, b1d, w2d, b2d, Wb, bb):
    sd, _ = _DT_MODES[_cfg("dt_mode", "f32r")]

    def padT(w, rows, cols):
        """Pad w.T (fp32/64 in) to [rows, cols], convert to stationary dtype."""
        out = np.zeros((rows, cols), np.float32)
        wt = np.asarray(w, np.float64).T
        out[: wt.shape[0], : wt.shape[1]] = wt.astype(np.float32)
        return _to_dt(out, sd)

    # fuse Wb into head layer 2 (float64 on host):
    # head @ Wb = relu(x@w1h.T+b1h) @ (w2h.T @ Wb) + (b2h @ Wb)
    wf = np.asarray(w2h, np.float64).T @ np.asarray(Wb, np.float64)  # [arc1, arc2]
    bf = np.asarray(b2h, np.float64) @ np.asarray(Wb, np.float64)  # [arc2]
    # carry the bilinear bias bb through arc pad row 500: headWT[500,:] = bb
    # (bias-only row: pad weight columns are zero), depT[500,:] = 1.0, so the
    # 512-wide scores contraction contributes bb * 1 exactly.
    bf = np.concatenate([bf, [float(np.asarray(bb).reshape(-1)[0])]])
    b2d_aug = np.concatenate([np.asarray(b2d, np.float64), [1.0]])
    return {
        "w1hT": padT(w1h, HIDDEN, ARC_P),
        "wfT": padT(wf.T, ARC_P, ARC_P),  # padT transposes back -> [arc1, arc2]
        "w1dT": padT(w1d, HIDDEN, ARC_P),
        "w2dT": padT(w2d, ARC_P, ARC_P),
        "biasesL": np.concatenate(
            [
                _bias_layout(b1h),
                _bias_layout(bf.astype(np.float32)),
                _bias_layout(b1d),
                _bias_layout(b2d_aug.astype(np.float32)),
                np.full((P, 1), float(np.asarray(bb).reshape(-1)[0]), np.float32),
            ],
            axis=1,
        ),
    }


def kernel(hidden_states, w1h, b1h, w2h, b2h, w1d, b1d, w2d, b2d, Wb, bb):
    import time

    _, md = _DT_MODES[_cfg("dt_mode", "f32r")]
    nc = _build(loop_n=int(_cfg("loop_n", 0)))
    shared = _prep_shared(w1h, b1h, w2h, b2h, w1d, b1d, w2d, b2d, Wb, bb)
    x = np.asarray(hidden_states, np.float32)
    in_maps = []
    for c in range(N_CORES):
        xc = x[c * B_PER_CORE : (c + 1) * B_PER_CORE].reshape(R, HIDDEN)
        in_maps.append({"xT": _to_dt(np.ascontiguousarray(xc.T), md), **shared})
    t0 = time.perf_counter()
    res = run_bass_kernel_spmd(nc, in_maps, core_ids=list(range(N_CORES)))
    _CACHE["last_run_seconds"] = time.perf_counter() - t0
    out = np.empty((BATCH, SEQ, SEQ), np.float32)
    for c in range(N_CORES):
        out[c * B_PER_CORE : (c + 1) * B_PER_CORE] = res.results[c]["scores"]
    return out


def _selftest():
    rng = np.random.default_rng(0)
    s_h = 1.0 / np.sqrt(HIDDEN)
    s_a = 1.0 / np.sqrt(ARC)
    ins = {
        "hidden_states": rng.standard_normal((BATCH, SEQ, HIDDEN)).astype(np.float32),
        "w1h": rng.uniform(-s_h, s_h, (ARC, HIDDEN)).astype(np.float32),
        "b1h": rng.uniform(-s_h, s_h, (ARC,)).astype(np.float32),
        "w2h": rng.uniform(-s_a, s_a, (ARC, ARC)).astype(np.float32),
        "b2h": rng.uniform(-s_a, s_a, (ARC,)).astype(np.float32),
        "w1d": rng.uniform(-s_h, s_h, (ARC, HIDDEN)).astype(np.float32),
        "b1d": rng.uniform(-s_h, s_h, (ARC,)).astype(np.float32),
        "w2d": rng.uniform(-s_a, s_a, (ARC, ARC)).astype(np.float32),
        "b2d": rng.uniform(-s_a, s_a, (ARC,)).astype(np.float32),
        "Wb": rng.uniform(-s_a, s_a, (ARC, ARC)).astype(np.float32),
        "bb": rng.uniform(-s_a, s_a, (1,)).astype(np.float32),
    }
    out = kernel(**ins)

    def ref_mlp(x, w1, b1, w2, b2):
        return np.maximum(x @ w1.T + b1, 0.0) @ w2.T + b2

    head = ref_mlp(ins["hidden_states"], ins["w1h"], ins["b1h"], ins["w2h"], ins["b2h"])
    dep = ref_mlp(ins["hidden_states"], ins["w1d"], ins["b1d"], ins["w2d"], ins["b2d"])
    headW = head @ ins["Wb"]
    exp = np.einsum("bia,bja->bij", headW, dep) + ins["bb"][0]
    err = np.abs(out - exp)
    rel = err.max() / np.abs(exp).max()
    print(f"max abs err {err.max():.3e}  absmax-rel {rel:.3e}")
    print(f"run seconds: {_CACHE.get('last_run_seconds'):.3f}")


if __name__ == "__main__":
    for mode in sys.argv[1:] or ["f32r"]:
        _CACHE.clear()
        _CACHE["dt_mode"] = mode
        print(f"--- dt_mode={mode}")
        _selftest()


# revision 23
# speedup vs baseline: 1.4216x; 1.4216x over previous
"""BiaffineAttention TRN2 kernel.

Full-input contract: kernel(**inputs) takes the unsharded reference inputs
(hidden_states [16,512,1024] f32 + MLP/bilinear params) and returns the full
arc_scores [16,512,512] f32.

Strategy:
- Data-parallel over batch across 8 NeuronCores (2 batches/core).
- All on-chip compute is feature-major (arc/hidden on partitions), so every
  matmul has its contraction dim on partitions and there are no on-chip
  transposes: the host passes x pre-transposed per core and weights
  pre-transposed + zero-padded (arc 500 -> 512).
- The bilinear weight Wb is fused into the head MLP's second layer on the
  host (Wf = w2h.T @ Wb, bf = b2h @ Wb, in float64), removing a whole
  [500x500] GEMM stage from the device.
- Matmuls run in float16 (same 10-bit mantissa as tf32, ~5e-4 relative
  error, but 2-byte operands at full PE issue rate with pipelined weight
  loads). float32r/bf16 variants remain selectable via _CACHE for testing.
"""

import sys

if "/opt/trn_rl_repo" not in sys.path:
    sys.path.insert(0, "/opt/trn_rl_repo")

import numpy as np

import concourse.bacc as bacc
import concourse.mybir as mybir
import concourse.tile as tile
from concourse.bass_utils import run_bass_kernel_spmd

N_CORES = 8
BATCH = 16
SEQ = 512
HIDDEN = 1024
ARC = 500
ARC_P = 512  # arc padded to a multiple of 128

P = 128
B_PER_CORE = BATCH // N_CORES  # 2
R = B_PER_CORE * SEQ  # 1024 rows per core
HK = HIDDEN // P  # 8 hidden k-tiles
AK = ARC_P // P  # 4 arc tiles
RC = R // SEQ  # 2 row chunks of 512

F32 = mybir.dt.float32
F32R = mybir.dt.float32r
BF16 = mybir.dt.bfloat16
F16 = mybir.dt.float16
AF = mybir.ActivationFunctionType

# matmul operand dtypes: (stationary/weight side, moving/activation side)
_DT_MODES = {
    "f32r": (F32R, F32R),
    "bf16": (BF16, BF16),
    "fp16": (F16, F16),
    "mixed": (BF16, F32R),
}
# max moving-operand width: 512 for 4-byte dtypes, 1024 for 2-byte
_MOV_W = {"f32r": 512, "bf16": 512, "fp16": 512, "mixed": 512}

_CACHE = {}


_DEFAULTS = {"dt_mode": "fp16"}


def _cfg(name, default):
    return _CACHE.get(name, _DEFAULTS.get(name, default))


def _emit(nc, tc, aps, loop_n=0):
    import contextlib

    mode = _cfg("dt_mode", "f32r")
    sd, md = _DT_MODES[mode]
    # scores-phase dtypes (both operands are on-chip activations)
    ssd = md if _cfg("scores_f32r", True) else sd
    smd = md

    ctx = contextlib.ExitStack()
    with ctx:
        cpool = ctx.enter_context(tc.tile_pool(name="const", bufs=1))
        apool = ctx.enter_context(tc.tile_pool(name="acts", bufs=1))
        pspool = ctx.enter_context(
            tc.tile_pool(name="psum", bufs=_cfg("ps_bufs", 8), space="PSUM")
        )
        opool = ctx.enter_context(tc.tile_pool(name="outs", bufs=8))

        # ---- constant loads, split per k-tile and ordered by first use so the
        # first matmul group starts as soon as its own slices land
        xT = cpool.tile([P, HK, R], md, tag="xT")
        w1h = cpool.tile([P, HK, ARC_P], sd, tag="w1h")
        w1d = cpool.tile([P, HK, ARC_P], sd, tag="w1d")
        xT_src = aps["xT"].rearrange("(ko p) r -> p ko r", p=P)
        w1h_src = aps["w1hT"].rearrange("(ko p) a -> p ko a", p=P)
        w1d_src = aps["w1dT"].rearrange("(ko p) a -> p ko a", p=P)
        for k in range(HK):
            (nc.sync if k == 0 else nc.gpsimd).dma_start(w1h[:, k], w1h_src[:, k])
        for rc in range(RC):
            for k in range(HK):
                eng = nc.scalar if k % 2 == 0 else nc.sync
                eng.dma_start(
                    xT[:, k, rc * SEQ : (rc + 1) * SEQ],
                    xT_src[:, k, rc * SEQ : (rc + 1) * SEQ],
                )
        biases = cpool.tile([P, 4 * AK + 1], F32, tag="biases")
        nc.gpsimd.dma_start(biases[:], aps["biasesL"])
        b1h = biases[:, 0 * AK : 1 * AK]
        bfh = biases[:, 1 * AK : 2 * AK]
        b1d = biases[:, 2 * AK : 3 * AK]
        b2d = biases[:, 3 * AK : 4 * AK]
        for k in range(HK):
            nc.gpsimd.dma_start(w1d[:, k], w1d_src[:, k])
        wf = cpool.tile([P, AK, ARC_P], sd, tag="wf")
        nc.gpsimd.dma_start(wf[:], aps["wfT"].rearrange("(ko p) a -> p ko a", p=P))
        w2d = cpool.tile([P, AK, ARC_P], sd, tag="w2d")
        nc.gpsimd.dma_start(w2d[:], aps["w2dT"].rearrange("(ko p) a -> p ko a", p=P))

        h1h = apool.tile([P, AK, R], md, tag="h1h")
        h1d = apool.tile([P, AK, R], md, tag="h1d")
        headWT = apool.tile([P, AK, R], ssd, tag="headWT")
        depT = apool.tile([P, AK, R], smd, tag="depT")

        def l1_half(w1, b1, h1, rc):
            # h1[:, :, rc-half] = relu(w1.T @ xT + b1); m-pairs share each
            # arriving xT k-tile (2 matmuls per 128KB of DMA feed)
            rs = slice(rc * SEQ, (rc + 1) * SEQ)
            for m0 in range(0, AK, 2):
                psa = pspool.tile([P, SEQ], F32, tag="ps", name=f"l1a_{rc}_{m0}")
                psb = pspool.tile([P, SEQ], F32, tag="ps", name=f"l1b_{rc}_{m0}")
                for k in range(HK):
                    for m, ps in ((m0, psa), (m0 + 1, psb)):
                        nc.tensor.matmul(
                            ps[:],
                            w1[:, k, m * P : (m + 1) * P],
                            xT[:, k, rs],
                            start=(k == 0),
                            stop=(k == HK - 1),
                        )
                for m, ps in ((m0, psa), (m0 + 1, psb)):
                    nc.scalar.activation(
                        h1[:, m, rs], ps[:], AF.Relu, bias=b1[:, m : m + 1]
                    )

        def l2_group(w2, b2, h1, outbuf, rc, m, epi):
            # outbuf[:, m, rc-half] = w2.T @ h1 + b2 at 512 width
            rs = slice(rc * SEQ, (rc + 1) * SEQ)
            ps = pspool.tile([P, SEQ], F32, tag="ps", name=f"ps2l_{rc}_{m}")
            for k in range(AK):
                nc.tensor.matmul(
                    ps[:],
                    w2[:, k, m * P : (m + 1) * P],
                    h1[:, k, rs],
                    start=(k == 0),
                    stop=(k == AK - 1),
                )
            if epi == "dve":
                nc.vector.tensor_tensor(
                    outbuf[:, m, rs],
                    ps[:],
                    b2[:, m : m + 1].to_broadcast((P, SEQ)),
                    mybir.AluOpType.add,
                )
            else:
                nc.scalar.activation(
                    outbuf[:, m, rs], ps[:], AF.Identity, bias=b2[:, m : m + 1]
                )

        def scores_kchunk(b, k, pss):
            # one k-slice of the scores accumulation: needs only the m=k tiles
            # of headWT/depT, so it can sit right after the m=k+1 layer-2 pair
            js = slice(b * SEQ, (b + 1) * SEQ)
            for i in range(AK):
                nc.tensor.matmul(
                    pss[i][:],
                    headWT[:, k, b * SEQ + i * P : b * SEQ + (i + 1) * P],
                    depT[:, k, js],
                    start=(k == 0),
                    stop=(k == AK - 1),
                )

        def scores_out(b, pss):
            # the bilinear bias bb is carried by arc pad row 500
            # (headWT[500,:] = bb via the fused bias, depT[500,:] = 1.0), so
            # the PSUM result is final: plain copy out, alternating engines
            for i in range(AK):
                ot = opool.tile([P, SEQ], F32, tag="scout")
                if i % 2 == 0:
                    nc.vector.tensor_copy(ot[:], pss[i][:])
                else:
                    # Identity (not Copy) keeps the ACT function table unchanged
                    # from the depT bias-adds -- table reloads are expensive
                    nc.scalar.activation(ot[:], pss[i][:], AF.Identity)
                eng = nc.sync if i % 2 == 0 else nc.scalar
                eng.dma_start(aps["scores"][b, i * P : (i + 1) * P, :], ot[:])

        if loop_n:
            hints = _cfg("loop_hints", ())
            if hints == "all":
                hints = tuple(
                    mybir.EngineType(e)
                    for e in ("PE", "Activation", "DVE", "SP", "Pool")
                )
            loop_cm = tc.For_i(0, loop_n, 1, hint_engines=hints)
        else:
            loop_cm = contextlib.nullcontext()
        if _cfg("tiny_body", False) and loop_n:
            with loop_cm:
                tb = apool.tile([P, 16], F32, tag="tinybody")
                nc.vector.tensor_copy(tb[:], biases[:, 0:16])
            return
        with loop_cm:
            # both layer-1s first (independent), so layer-2 never starves PE;
            # rc-halves in DMA-arrival order
            l1_half(w1h, b1h, h1h, 0)
            l1_half(w1d, b1d, h1d, 0)
            l1_half(w1h, b1h, h1h, 1)
            l1_half(w1d, b1d, h1d, 1)
            # layer 2 + scores software-pipelined per batch-half: the scores
            # k-chunk for m=k is emitted after the m=k+1 layer-2 pair, so the
            # in-order PE stream never waits on an epilogue drain.
            for rc in range(B_PER_CORE):
                pss = []
                for i in range(AK):
                    ps = pspool.tile([P, SEQ], F32, tag="ps", name=f"ps2s_{rc}_{i}")
                    pss.append(ps)
                for m in range(AK):
                    l2_group(wf, bfh, h1h, headWT, rc, m, "dve")
                    l2_group(w2d, b2d, h1d, depT, rc, m, "act")
                    if m >= 1:
                        scores_kchunk(rc, m - 1, pss)
                scores_kchunk(rc, AK - 1, pss)
                scores_out(rc, pss)


def _build(loop_n=0):
    sd, md = _DT_MODES[_cfg("dt_mode", "f32r")]
    key = ("nc", _cfg("dt_mode", "f32r"), _cfg("scores_f32r", True), loop_n, _cfg("loop_hints", ()), _cfg("tiny_body", False))
    if key in _CACHE:
        return _CACHE[key]
    nc = bacc.Bacc("TRN2", target_bir_lowering=False, debug=False, num_devices=N_CORES)

    def dram(name, shape, dt):
        return nc.dram_tensor(name, shape, dt, kind="ExternalInput").ap()

    aps = {
        "xT": dram("xT", [HIDDEN, R], md),
        "w1hT": dram("w1hT", [HIDDEN, ARC_P], sd),
        "wfT": dram("wfT", [ARC_P, ARC_P], sd),
        "w1dT": dram("w1dT", [HIDDEN, ARC_P], sd),
        "w2dT": dram("w2dT", [ARC_P, ARC_P], sd),
        "biasesL": dram("biasesL", [P, 4 * AK + 1], F32),
        "scores": nc.dram_tensor(
            "scores", [B_PER_CORE, SEQ, SEQ], F32, kind="ExternalOutput"
        ).ap(),
    }
    with tile.TileContext(nc) as tc:
        _emit(nc, tc, aps, loop_n=loop_n)
    nc.compile()
    _CACHE[key] = nc
    return nc


def _round_tf32(a):
    """fp32 -> tf32 (10-bit mantissa) RNE, returned as fp32 bits."""
    b = np.ascontiguousarray(a, np.float32).view(np.uint32).copy()
    lsb = (b >> 13) & 1
    b += 0x0FFF + lsb
    b &= np.uint32(0xFFFFE000)
    return b.view(np.float32)


def _to_dt(a, dt):
    """Convert fp32 ndarray to the numpy repr of mybir dtype dt."""
    if dt == F32R:
        return _round_tf32(a)
    if dt == BF16:
        import ml_dtypes

        return np.asarray(a, np.float32).astype(ml_dtypes.bfloat16)
    if dt == F16:
        return np.asarray(a, np.float32).astype(np.float16)
    return np.asarray(a, np.float32)


def _bias_layout(b):
    """[ARC] (unpadded) -> [128, AK] with arc index = col*128 + partition."""
    bp = np.zeros(ARC_P, np.float32)
    b = np.asarray(b, np.float32)
    bp[: b.shape[0]] = b
    return np.ascontiguousarray(bp.reshape(AK, P).T)


def _prep_shared(w1h, b1h, w2h, b2h, w1d, b1d, w2d, b2d, Wb, bb):
    sd, _ = _DT_MODES[_cfg("dt_mode", "f32r")]

    def padT(w, rows, cols):
        """Pad w.T (fp32/64 in) to [rows, cols], convert to stationary dtype."""
        out = np.zeros((rows, cols), np.float32)
        wt = np.asarray(w, np.float64).T
        out[: wt.shape[0], : wt.shape[1]] = wt.astype(np.float32)
        return _to_dt(out, sd)

    # fuse Wb into head layer 2 (float64 on host):
    # head @ Wb = relu(x@w1h.T+b1h) @ (w2h.T @ Wb) + (b2h @ Wb)
    wf = np.asarray(w2h, np.float64).T @ np.asarray(Wb, np.float64)  # [arc1, arc2]
    bf = np.asarray(b2h, np.float64) @ np.asarray(Wb, np.float64)  # [arc2]
    # carry the bilinear bias bb through arc pad row 500: headWT[500,:] = bb
    # (bias-only row: pad weight columns are zero), depT[500,:] = 1.0, so the
    # 512-wide scores contraction contributes bb * 1 exactly.
    bf = np.concatenate([bf, [float(np.asarray(bb).reshape(-1)[0])]])
    b2d_aug = np.concatenate([np.asarray(b2d, np.float64), [1.0]])
    return {
        "w1hT": padT(w1h, HIDDEN, ARC_P),
        "wfT": padT(wf.T, ARC_P, ARC_P),  # padT transposes back -> [arc1, arc2]
        "w1dT": padT(w1d, HIDDEN, ARC_P),
        "w2dT": padT(w2d, ARC_P, ARC_P),
        "biasesL": np.concatenate(
            [
                _bias_layout(b1h),
                _bias_layout(bf.astype(np.float32)),
                _bias_layout(b1d),
                _bias_layout(b2d_aug.astype(np.float32)),
                np.full((P, 1), float(np.asarray(bb).reshape(-1)[0]), np.float32),
            ],
            axis=1,
        ),
    }


def kernel(hidden_states, w1h, b1h, w2h, b2h, w1d, b1d, w2d, b2d, Wb, bb):
    import time

    _, md = _DT_MODES[_cfg("dt_mode", "f32r")]
    nc = _build(loop_n=int(_cfg("loop_n", 0)))
    shared = _prep_shared(w1h, b1h, w2h, b2h, w1d, b1d, w2d, b2d, Wb, bb)
    x = np.asarray(hidden_states, np.float32)
    in_maps = []
    for c in range(N_CORES):
        xc = x[c * B_PER_CORE : (c + 1) * B_PER_CORE].reshape(R, HIDDEN)
        in_maps.append({"xT": _to_dt(np.ascontiguousarray(xc.T), md), **shared})
    t0 = time.perf_counter()
    res = run_bass_kernel_spmd(nc, in_maps, core_ids=list(range(N_CORES)))
    _CACHE["last_run_seconds"] = time.perf_counter() - t0
    out = np.empty((BATCH, SEQ, SEQ), np.float32)
    for c in range(N_CORES):
        out[c * B_PER_CORE : (c + 1) * B_PER_CORE] = res.results[c]["scores"]
    return out


def _selftest():
    rng = np.random.default_rng(0)
    s_h = 1.0 / np.sqrt(HIDDEN)
    s_a = 1.0 / np.sqrt(ARC)
    ins = {
        "hidden_states": rng.standard_normal((BATCH, SEQ, HIDDEN)).astype(np.float32),
        "w1h": rng.uniform(-s_h, s_h, (ARC, HIDDEN)).astype(np.float32),
        "b1h": rng.uniform(-s_h, s_h, (ARC,)).astype(np.float32),
        "w2h": rng.uniform(-s_a, s_a, (ARC, ARC)).astype(np.float32),
        "b2h": rng.uniform(-s_a, s_a, (ARC,)).astype(np.float32),
        "w1d": rng.uniform(-s_h, s_h, (ARC, HIDDEN)).astype(np.float32),
        "b1d": rng.uniform(-s_h, s_h, (ARC,)).astype(np.float32),
        "w2d": rng.uniform(-s_a, s_a, (ARC, ARC)).astype(np.float32),
        "b2d": rng.uniform(-s_a, s_a, (ARC,)).astype(np.float32),
        "Wb": rng.uniform(-s_a, s_a, (ARC, ARC)).astype(np.float32),
        "bb": rng.uniform(-s_a, s_a, (1,)).astype(np.float32),
    }
    out = kernel(**ins)

    def ref_mlp(x, w1, b1, w2, b2):
        return np.maximum(x @ w1.T + b1, 0.0) @ w2.T + b2

    head = ref_mlp(ins["hidden_states"], ins["w1h"], ins["b1h"], ins["w2h"], ins["b2h"])
    dep = ref_mlp(ins["hidden_states"], ins["w1d"], ins["b1d"], ins["w2d"], ins["b2d"])
    headW = head @ ins["Wb"]
    exp = np.einsum("bia,bja->bij", headW, dep) + ins["bb"][0]
    err = np.abs(out - exp)
    rel = err.max() / np.abs(exp).max()
    print(f"max abs err {err.max():.3e}  absmax-rel {rel:.3e}")
    print(f"run seconds: {_CACHE.get('last_run_seconds'):.3f}")


if __name__ == "__main__":
    for mode in sys.argv[1:] or ["fp16"]:
        _CACHE.clear()
        _CACHE["dt_mode"] = mode
        print(f"--- dt_mode={mode}")
        _selftest()
